# revision 4
# baseline (speedup 1.0000x reference)
"""Trainium2 Bass kernel for nn_AutoCorrelation (8 NeuronCores, data-parallel over batch).

Algorithm (reference: AutoCorrelation block):
  corr = irfft(rfft(q, L) * conj(rfft(k, L)))        # circular cross-correlation
  top-6 delays from batch-mean of corr (mean over H,E then N)
  out  = sum_k softmax(mean[:, idx])_k * roll(v, -idx_k)

Implementation:
  - FFTs become dense DFT matmuls on the TensorEngine: rfft -> q @ C and
    q @ Sm with C[l,f]=cos(2*pi*l*f/L), Sm[l,f]=-sin(...), f=0..511, and the
    Nyquist (f=512) cosine column packed into Sm[:,0] (sin column there is 0).
    irfft -> Pre @ A + Pim @ B with the matching inverse rows (A[0]=DC row,
    B[0]=Nyquist row).
  - Phase 1 kernel (per core, 4 batch items): forward DFTs, pointwise complex
    product (VectorE), inverse DFT, corr written to HBM, per-delay row-sums
    reduced for the top-k statistic.
  - Host: tiny (N,L) mean -> top-6 indices + softmax weights.
  - Phase 2 kernel: out = sum_k w*roll(v) as PSUM-accumulated matmuls with
    w-scaled shifted-identity stationary matrices (shift along L = partition
    permutation, contracted on the TensorEngine).
"""
import math
import sys

sys.path.insert(0, "/opt/trn_rl_repo")

import numpy as np
import ml_dtypes

import concourse.bass as bass
import concourse.tile as tile
from concourse import bacc, mybir
from concourse.bass import ts
from concourse.bass_utils import run_bass_kernel_spmd

_dt = mybir.dt

N, L, H, E = 32, 1024, 8, 64
R = H * E                 # 512 rows (h,e) per batch item
NCORES = 8
NLOC = N // NCORES        # 4 batch items per core
F = 512                   # packed rfft freqs (f=0..511; Nyquist in slot 0)
TOPK = int(1.0 * math.log(L))  # 6
LB = L // 128             # 8 l/tau blocks
FB = F // 128             # 4 f blocks

# phase-1 matmul dtype: "f32r" (full fp32 precision at ~bf16 rate) or "bf16"
P1_MODE = "bf16"
TRACE = [False]           # test.py flips this to collect exec_time_ns
LAST_EXEC_NS = [0, 0]     # phase1, phase2 exec time (when TRACE)


def _dft_mats():
    l = np.arange(L)[:, None].astype(np.float64)
    f = np.arange(F)[None, :].astype(np.float64)
    ang = 2.0 * np.pi * l * f / L
    C = np.cos(ang)
    Sm = -np.sin(ang)
    Sm[:, 0] = (-1.0) ** np.arange(L)
    t = np.arange(L)[None, :].astype(np.float64)
    fa = np.arange(F)[:, None].astype(np.float64)
    ang2 = 2.0 * np.pi * fa * t / L
    A = (2.0 / L) * np.cos(ang2)
    B = -(2.0 / L) * np.sin(ang2)
    A[0, :] = 1.0 / L
    B[0, :] = (1.0 / L) * ((-1.0) ** np.arange(L))
    return C, Sm, A, B


def _build_phase1(mode):
    store = _dt.float32 if mode == "f32r" else _dt.bfloat16
    mmdt = _dt.float32r if mode == "f32r" else _dt.bfloat16

    nc = bacc.Bacc("TRN2", target_bir_lowering=False, debug=False,
                   num_devices=NCORES)
    q_d = nc.dram_tensor("q", [NLOC, L, R], store, kind="ExternalInput").ap()
    k_d = nc.dram_tensor("k", [NLOC, L, R], store, kind="ExternalInput").ap()
    c_d = nc.dram_tensor("cmat", [L, F], store, kind="ExternalInput").ap()
    s_d = nc.dram_tensor("smat", [L, F], store, kind="ExternalInput").ap()
    a_d = nc.dram_tensor("amat", [F, L], store, kind="ExternalInput").ap()
    b_d = nc.dram_tensor("bmat", [F, L], store, kind="ExternalInput").ap()
    corr_d = nc.dram_tensor("corr", [NLOC, L, R], _dt.float32,
                            kind="ExternalOutput").ap()
    msum_d = nc.dram_tensor("msum", [NLOC, 128, LB], _dt.float32,
                            kind="ExternalOutput").ap()

    def mm(ps, lhsT, rhs, start, stop):
        nc.tensor.matmul(ps, lhsT.bitcast(mmdt), rhs.bitcast(mmdt),
                         start=start, stop=stop)

    with tile.TileContext(nc) as tc:
        with tc.tile_pool(name="const", bufs=1) as cp, \
             tc.tile_pool(name="qk", bufs=12) as qk, \
             tc.tile_pool(name="pp", bufs=6) as pp, \
             tc.tile_pool(name="tmp", bufs=2) as tp, \
             tc.tile_pool(name="out", bufs=4) as op, \
             tc.tile_pool(name="ps", bufs=6, space="PSUM") as psf, \
             tc.tile_pool(name="psi", bufs=2, space="PSUM") as psi:

            cs, ss = [], []
            for lb in range(LB):
                t = cp.tile([128, F], store, tag=f"c{lb}")
                nc.sync.dma_start(t[:], c_d[ts(lb, 128), :])
                cs.append(t)
                t = cp.tile([128, F], store, tag=f"s{lb}")
                nc.sync.dma_start(t[:], s_d[ts(lb, 128), :])
                ss.append(t)
            am, bm = [], []
            for fb in range(FB):
                t = cp.tile([128, L], store, tag=f"a{fb}")
                nc.sync.dma_start(t[:], a_d[ts(fb, 128), :])
                am.append(t)
                t = cp.tile([128, L], store, tag=f"b{fb}")
                nc.sync.dma_start(t[:], b_d[ts(fb, 128), :])
                bm.append(t)

            for n in range(NLOC):
                q_sb, k_sb = [], []
                for lb in range(LB):
                    t = qk.tile([128, R], store, tag="q")
                    nc.sync.dma_start(t[:], q_d[n, ts(lb, 128), :])
                    q_sb.append(t)
                    t = qk.tile([128, R], store, tag="k")
                    nc.sync.dma_start(t[:], k_d[n, ts(lb, 128), :])
                    k_sb.append(t)

                pre_sb, pim_sb = [], []
                for fb in range(FB):
                    ps_qre = psf.tile([128, R], _dt.float32, tag="fwd")
                    ps_qim = psf.tile([128, R], _dt.float32, tag="fwd")
                    ps_kre = psf.tile([128, R], _dt.float32, tag="fwd")
                    ps_kim = psf.tile([128, R], _dt.float32, tag="fwd")
                    for lb in range(LB):
                        mm(ps_qre[:], cs[lb][:, ts(fb, 128)], q_sb[lb][:],
                           lb == 0, lb == LB - 1)
                    for lb in range(LB):
                        mm(ps_qim[:], ss[lb][:, ts(fb, 128)], q_sb[lb][:],
                           lb == 0, lb == LB - 1)
                    for lb in range(LB):
                        mm(ps_kre[:], cs[lb][:, ts(fb, 128)], k_sb[lb][:],
                           lb == 0, lb == LB - 1)
                    for lb in range(LB):
                        mm(ps_kim[:], ss[lb][:, ts(fb, 128)], k_sb[lb][:],
                           lb == 0, lb == LB - 1)

                    qre = tp.tile([128, R], _dt.float32, tag="qre")
                    qim = tp.tile([128, R], _dt.float32, tag="qim")
                    nc.scalar.mul(qre[:], ps_qre[:], 1.0)
                    nc.scalar.mul(qim[:], ps_qim[:], 1.0)
                    t1 = tp.tile([128, R], _dt.float32, tag="t1")
                    t2 = tp.tile([128, R], _dt.float32, tag="t2")
                    nc.vector.tensor_mul(t1[:], qre[:], ps_kre[:])
                    nc.vector.tensor_mul(t2[:], qim[:], ps_kim[:])
                    pre = pp.tile([128, R], store, tag="pre")
                    nc.vector.tensor_add(pre[:], t1[:], t2[:])
                    t3 = tp.tile([128, R], _dt.float32, tag="t3")
                    t4 = tp.tile([128, R], _dt.float32, tag="t4")
                    nc.vector.tensor_mul(t3[:], qim[:], ps_kre[:])
                    nc.vector.tensor_mul(t4[:], qre[:], ps_kim[:])
                    pim = pp.tile([128, R], store, tag="pim")
                    nc.vector.tensor_sub(pim[:], t3[:], t4[:])
                    if fb == 0:
                        # slot-0 carries DC (cos col) and Nyquist (packed in
                        # Sm col 0): Pre[0] = Qre0*Kre0 (= t1 row0),
                        # Pim[0] = Qim0*Kim0 (= t2 row0)
                        nc.vector.tensor_copy(pre[0:1, :], t1[0:1, :])
                        nc.vector.tensor_copy(pim[0:1, :], t2[0:1, :])
                    pre_sb.append(pre)
                    pim_sb.append(pim)

                mean_sb = op.tile([128, LB], _dt.float32, tag="mean")
                for tb in range(LB):
                    ps_c = psi.tile([128, R], _dt.float32, tag="inv")
                    first = True
                    for fb in range(FB):
                        mm(ps_c[:], am[fb][:, ts(tb, 128)], pre_sb[fb][:],
                           first, False)
                        first = False
                        mm(ps_c[:], bm[fb][:, ts(tb, 128)], pim_sb[fb][:],
                           False, fb == FB - 1)
                    corr_sb = op.tile([128, R], _dt.float32, tag="corr")
                    nc.vector.tensor_copy(corr_sb[:], ps_c[:])
                    nc.sync.dma_start(corr_d[n, ts(tb, 128), :], corr_sb[:])
                    nc.vector.tensor_reduce(
                        mean_sb[:, tb:tb + 1], corr_sb[:],
                        axis=mybir.AxisListType.X, op=mybir.AluOpType.add)
                nc.sync.dma_start(msum_d[n][:], mean_sb[:])
    nc.compile()
    return nc


def _build_phase2(entries):
    """entries: per output block b, list of (src_block, seg_idx); seg_idx
    indexes the g stationaries tensor (NLOC, NSEG, 128, 128)."""
    nseg = max(si for segs in entries for _, si in segs) + 1
    nc = bacc.Bacc("TRN2", target_bir_lowering=False, debug=False,
                   num_devices=NCORES)
    v_d = nc.dram_tensor("v", [NLOC, L, R], _dt.bfloat16,
                         kind="ExternalInput").ap()
    g_d = nc.dram_tensor("g", [NLOC, nseg, 128, 128], _dt.bfloat16,
                         kind="ExternalInput").ap()
    out_d = nc.dram_tensor("out", [NLOC, L, R], _dt.float32,
                           kind="ExternalOutput").ap()

    with tile.TileContext(nc) as tc:
        with tc.tile_pool(name="v", bufs=12) as vp, \
             tc.tile_pool(name="g", bufs=2 * nseg) as gp, \
             tc.tile_pool(name="o", bufs=4) as op, \
             tc.tile_pool(name="ps", bufs=4, space="PSUM") as psp:
            for n in range(NLOC):
                v_sb = []
                for a in range(LB):
                    t = vp.tile([128, R], _dt.bfloat16, tag="v")
                    nc.sync.dma_start(t[:], v_d[n, ts(a, 128), :])
                    v_sb.append(t)
                g_sb = []
                for si in range(nseg):
                    t = gp.tile([128, 128], _dt.bfloat16, tag="g")
                    nc.sync.dma_start(t[:], g_d[n, si][:])
                    g_sb.append(t)
                for b in range(LB):
                    segs = entries[b]
                    ps = psp.tile([128, R], _dt.float32, tag="ps")
                    for i, (a, si) in enumerate(segs):
                        nc.tensor.matmul(ps[:], g_sb[si][:], v_sb[a][:],
                                         start=(i == 0),
                                         stop=(i == len(segs) - 1))
                    o_sb = op.tile([128, R], _dt.float32, tag="o")
                    nc.vector.tensor_copy(o_sb[:], ps[:])
                    nc.sync.dma_start(out_d[n, ts(b, 128), :], o_sb[:])
    nc.compile()
    return nc


_P1_CACHE = {}


def _phase1_nc(mode):
    if mode not in _P1_CACHE:
        _P1_CACHE[mode] = _build_phase1(mode)
    return _P1_CACHE[mode]


def _run(nc, in_maps, phase):
    res = run_bass_kernel_spmd(nc, in_maps, core_ids=list(range(NCORES)),
                               trace=TRACE[0])
    if TRACE[0]:
        LAST_EXEC_NS[phase] = res.exec_time_ns
    return res.results


def kernel(queries, keys, values):
    queries = np.ascontiguousarray(np.asarray(queries, dtype=np.float32))
    keys = np.ascontiguousarray(np.asarray(keys, dtype=np.float32))
    values = np.ascontiguousarray(np.asarray(values, dtype=np.float32))

    mode = P1_MODE
    store_np = np.float32 if mode == "f32r" else ml_dtypes.bfloat16
    C, Sm, A, B = _dft_mats()
    Cs = np.ascontiguousarray(C, dtype=np.float32).astype(store_np)
    Ss = np.ascontiguousarray(Sm, dtype=np.float32).astype(store_np)
    As = np.ascontiguousarray(A, dtype=np.float32).astype(store_np)
    Bs = np.ascontiguousarray(B, dtype=np.float32).astype(store_np)

    q3 = queries.reshape(N, L, R)
    k3 = keys.reshape(N, L, R)
    v3 = values.reshape(N, L, R)

    nc1 = _phase1_nc(mode)
    in_maps = []
    for c in range(NCORES):
        sl = slice(c * NLOC, (c + 1) * NLOC)
        in_maps.append({
            "q": q3[sl].astype(store_np),
            "k": k3[sl].astype(store_np),
            "cmat": Cs, "smat": Ss, "amat": As, "bmat": Bs,
        })
    res1 = _run(nc1, in_maps, 0)

    corr = np.concatenate([r["corr"] for r in res1], axis=0)  # (N, L, R) f32
    msum = np.concatenate([r["msum"] for r in res1], axis=0)  # (N, 128, LB)
    mean = msum.transpose(0, 2, 1).reshape(N, L) / R          # (N, L)

    g = mean.mean(axis=0)
    idx = np.argsort(-g, kind="stable")[:TOPK]
    w = mean[:, idx]
    e = np.exp(w - w.max(axis=1, keepdims=True))
    w = (e / e.sum(axis=1, keepdims=True)).astype(np.float32)  # (N, TOPK)

    # phase-2 stationaries: out[b*128+j] += w_k * v[(b*128+j+idx_k) mod L]
    # merged per (b, src_block); matrix content is b-independent, so dedup
    # identical segment sets across b.
    seg_of = {}
    pat = []
    entries = [[] for _ in range(LB)]
    for b in range(LB):
        acc = {}
        for kk in range(TOPK):
            sh = int(idx[kk])
            r = sh % 128
            a = ((b * 128 + sh) // 128) % LB
            acc.setdefault(a, []).append(("d1", r, kk))
            if r > 0:
                acc.setdefault((a + 1) % LB, []).append(("d2", r, kk))
        for a, parts in sorted(acc.items()):
            key = tuple(sorted(parts))
            if key not in seg_of:
                seg_of[key] = len(pat)
                pat.append(parts)
            entries[b].append((a, seg_of[key]))
    nseg = len(pat)
    gmat = np.zeros((NLOC * NCORES, nseg, 128, 128), np.float32)
    jj = np.arange(128)
    for si, parts in enumerate(pat):
        for which, r, kk in parts:
            if which == "d1":
                j = jj[: 128 - r]
                gmat[:, si, j + r, j] += w[:, kk][:, None]
            else:
                j = jj[128 - r:]
                gmat[:, si, j - (128 - r), j] += w[:, kk][:, None]
    gmat = gmat.astype(ml_dtypes.bfloat16)

    nc2 = _build_phase2(entries)
    in_maps2 = []
    for c in range(NCORES):
        sl = slice(c * NLOC, (c + 1) * NLOC)
        in_maps2.append({
            "v": v3[sl].astype(ml_dtypes.bfloat16),
            "g": gmat[sl],
        })
    res2 = _run(nc2, in_maps2, 1)
    out = np.concatenate([r["out"] for r in res2], axis=0)    # (N, L, R)

    out_full = out.reshape(N, L, H, E).astype(np.float32)
    corr_full = corr.reshape(N, L, H, E).astype(np.float32)
    return out_full, corr_full


# revision 8
# speedup vs baseline: 1.0181x; 1.0181x over previous
"""Trainium2 Bass kernel for nn_AutoCorrelation (8 NeuronCores, data-parallel over batch).

Algorithm (reference: AutoCorrelation block):
  corr = irfft(rfft(q, L) * conj(rfft(k, L)))        # circular cross-correlation
  top-6 delays from batch-mean of corr (mean over H,E then N)
  out  = sum_k softmax(mean[:, idx])_k * roll(v, -idx_k)

Implementation:
  - FFTs become dense DFT matmuls on the TensorEngine: rfft -> q @ C and
    q @ Sm with C[l,f]=cos(2*pi*l*f/L), Sm[l,f]=-sin(...), f=0..511, and the
    Nyquist (f=512) cosine column packed into Sm[:,0] (sin column there is 0).
    irfft -> Pre @ A + Pim @ B with the matching inverse rows (A[0]=DC row,
    B[0]=Nyquist row).
  - Phase 1 kernel (per core, 4 batch items): forward DFTs, pointwise complex
    product (VectorE), inverse DFT, corr written to HBM, per-delay row-sums
    reduced for the top-k statistic.
  - Host: tiny (N,L) mean -> top-6 indices + softmax weights.
  - Phase 2 kernel: out = sum_k w*roll(v) as PSUM-accumulated matmuls with
    w-scaled shifted-identity stationary matrices (shift along L = partition
    permutation, contracted on the TensorEngine).
"""
import math
import sys

sys.path.insert(0, "/opt/trn_rl_repo")

import numpy as np
import ml_dtypes

import concourse.bass as bass
import concourse.tile as tile
from concourse import bacc, mybir
from concourse.bass import ts
from concourse.bass_utils import run_bass_kernel_spmd

_dt = mybir.dt

N, L, H, E = 32, 1024, 8, 64
R = H * E                 # 512 rows (h,e) per batch item
NCORES = 8
NLOC = N // NCORES        # 4 batch items per core
F = 512                   # packed rfft freqs (f=0..511; Nyquist in slot 0)
TOPK = int(1.0 * math.log(L))  # 6
LB = L // 128             # 8 l/tau blocks
FB = F // 128             # 4 f blocks

# phase-1 matmul dtype: "f32r" (full fp32 precision at ~bf16 rate) or "bf16"
P1_MODE = "bf16"
TRACE = [False]           # test.py flips this to collect exec_time_ns
LAST_EXEC_NS = [0, 0]     # phase1, phase2 exec time (when TRACE)


def _dft_mats():
    l = np.arange(L)[:, None].astype(np.float64)
    f = np.arange(F)[None, :].astype(np.float64)
    ang = 2.0 * np.pi * l * f / L
    C = np.cos(ang)
    Sm = -np.sin(ang)
    Sm[:, 0] = (-1.0) ** np.arange(L)
    t = np.arange(L)[None, :].astype(np.float64)
    fa = np.arange(F)[:, None].astype(np.float64)
    ang2 = 2.0 * np.pi * fa * t / L
    A = (2.0 / L) * np.cos(ang2)
    B = -(2.0 / L) * np.sin(ang2)
    A[0, :] = 1.0 / L
    B[0, :] = (1.0 / L) * ((-1.0) ** np.arange(L))
    return C, Sm, A, B


def _build_phase1(mode):
    store = _dt.float32 if mode == "f32r" else _dt.bfloat16
    mmdt = _dt.float32r if mode == "f32r" else _dt.bfloat16

    nc = bacc.Bacc("TRN2", target_bir_lowering=False, debug=False,
                   num_devices=NCORES)
    q_d = nc.dram_tensor("q", [NLOC, L, R], store, kind="ExternalInput").ap()
    k_d = nc.dram_tensor("k", [NLOC, L, R], store, kind="ExternalInput").ap()
    c_d = nc.dram_tensor("cmat", [L, F], store, kind="ExternalInput").ap()
    s_d = nc.dram_tensor("smat", [L, F], store, kind="ExternalInput").ap()
    a_d = nc.dram_tensor("amat", [F, L], store, kind="ExternalInput").ap()
    b_d = nc.dram_tensor("bmat", [F, L], store, kind="ExternalInput").ap()
    corr_d = nc.dram_tensor("corr", [NLOC, L, R], _dt.float32,
                            kind="ExternalOutput").ap()
    msum_d = nc.dram_tensor("msum", [NLOC, 128, LB], _dt.float32,
                            kind="ExternalOutput").ap()

    def mm(ps, lhsT, rhs, start, stop):
        nc.tensor.matmul(ps, lhsT.bitcast(mmdt), rhs.bitcast(mmdt),
                         start=start, stop=stop)

    with tile.TileContext(nc) as tc:
        with tc.tile_pool(name="const", bufs=1) as cp, \
             tc.tile_pool(name="qk", bufs=12) as qk, \
             tc.tile_pool(name="pp", bufs=6) as pp, \
             tc.tile_pool(name="tmp", bufs=2) as tp, \
             tc.tile_pool(name="out", bufs=4) as op, \
             tc.tile_pool(name="ps", bufs=6, space="PSUM") as psf, \
             tc.tile_pool(name="psi", bufs=2, space="PSUM") as psi:

            # head-latency-ordered loads: the first forward chain (q @ C)
            # only needs cs[lb] + q[lb], so stream those first, then k, then
            # the sin matrices; the inverse matrices are not needed until
            # after the first full fb loop.
            cs, ss, am, bm = [], [], [], []
            q0, k0 = [], []
            for lb in range(LB):
                t = cp.tile([128, F], store, tag=f"c{lb}")
                nc.sync.dma_start(t[:], c_d[ts(lb, 128), :])
                cs.append(t)
                t = qk.tile([128, R], store, tag="q")
                nc.sync.dma_start(t[:], q_d[0, ts(lb, 128), :])
                q0.append(t)
            for lb in range(LB):
                t = qk.tile([128, R], store, tag="k")
                nc.sync.dma_start(t[:], k_d[0, ts(lb, 128), :])
                k0.append(t)
            for lb in range(LB):
                t = cp.tile([128, F], store, tag=f"s{lb}")
                nc.sync.dma_start(t[:], s_d[ts(lb, 128), :])
                ss.append(t)
            for fb in range(FB):
                t = cp.tile([128, L], store, tag=f"a{fb}")
                nc.sync.dma_start(t[:], a_d[ts(fb, 128), :])
                am.append(t)
                t = cp.tile([128, L], store, tag=f"b{fb}")
                nc.sync.dma_start(t[:], b_d[ts(fb, 128), :])
                bm.append(t)

            for n in range(NLOC):
                if n == 0:
                    q_sb, k_sb = q0, k0
                else:
                    q_sb, k_sb = [], []
                    for lb in range(LB):
                        t = qk.tile([128, R], store, tag="q")
                        nc.sync.dma_start(t[:], q_d[n, ts(lb, 128), :])
                        q_sb.append(t)
                        t = qk.tile([128, R], store, tag="k")
                        nc.sync.dma_start(t[:], k_d[n, ts(lb, 128), :])
                        k_sb.append(t)

                pre_sb, pim_sb = [], []
                for fb in range(FB):
                    ps_qre = psf.tile([128, R], _dt.float32, tag="fwd")
                    ps_qim = psf.tile([128, R], _dt.float32, tag="fwd")
                    ps_kre = psf.tile([128, R], _dt.float32, tag="fwd")
                    ps_kim = psf.tile([128, R], _dt.float32, tag="fwd")
                    # chain order matches head DMA arrival: cs+q, cs+k, ss+q, ss+k
                    for lb in range(LB):
                        mm(ps_qre[:], cs[lb][:, ts(fb, 128)], q_sb[lb][:],
                           lb == 0, lb == LB - 1)
                    for lb in range(LB):
                        mm(ps_kre[:], cs[lb][:, ts(fb, 128)], k_sb[lb][:],
                           lb == 0, lb == LB - 1)
                    for lb in range(LB):
                        mm(ps_qim[:], ss[lb][:, ts(fb, 128)], q_sb[lb][:],
                           lb == 0, lb == LB - 1)
                    for lb in range(LB):
                        mm(ps_kim[:], ss[lb][:, ts(fb, 128)], k_sb[lb][:],
                           lb == 0, lb == LB - 1)

                    qre = tp.tile([128, R], _dt.float32, tag="qre")
                    qim = tp.tile([128, R], _dt.float32, tag="qim")
                    nc.scalar.mul(qre[:], ps_qre[:], 1.0)
                    nc.scalar.mul(qim[:], ps_qim[:], 1.0)
                    t1 = tp.tile([128, R], _dt.float32, tag="t1")
                    t2 = tp.tile([128, R], _dt.float32, tag="t2")
                    nc.vector.tensor_mul(t1[:], qre[:], ps_kre[:])
                    nc.vector.tensor_mul(t2[:], qim[:], ps_kim[:])
                    pre = pp.tile([128, R], store, tag="pre")
                    nc.vector.tensor_add(pre[:], t1[:], t2[:])
                    t3 = tp.tile([128, R], _dt.float32, tag="t3")
                    t4 = tp.tile([128, R], _dt.float32, tag="t4")
                    nc.vector.tensor_mul(t3[:], qim[:], ps_kre[:])
                    nc.vector.tensor_mul(t4[:], qre[:], ps_kim[:])
                    pim = pp.tile([128, R], store, tag="pim")
                    nc.vector.tensor_sub(pim[:], t3[:], t4[:])
                    if fb == 0:
                        # slot-0 carries DC (cos col) and Nyquist (packed in
                        # Sm col 0): Pre[0] = Qre0*Kre0 (= t1 row0),
                        # Pim[0] = Qim0*Kim0 (= t2 row0)
                        nc.vector.tensor_copy(pre[0:1, :], t1[0:1, :])
                        nc.vector.tensor_copy(pim[0:1, :], t2[0:1, :])
                    pre_sb.append(pre)
                    pim_sb.append(pim)

                mean_sb = op.tile([128, LB], _dt.float32, tag="mean")
                for tb in range(LB):
                    ps_c = psi.tile([128, R], _dt.float32, tag="inv")
                    first = True
                    for fb in range(FB):
                        mm(ps_c[:], am[fb][:, ts(tb, 128)], pre_sb[fb][:],
                           first, False)
                        first = False
                        mm(ps_c[:], bm[fb][:, ts(tb, 128)], pim_sb[fb][:],
                           False, fb == FB - 1)
                    corr_sb = op.tile([128, R], _dt.float32, tag="corr")
                    nc.vector.tensor_copy(corr_sb[:], ps_c[:])
                    nc.sync.dma_start(corr_d[n, ts(tb, 128), :], corr_sb[:])
                    nc.vector.tensor_reduce(
                        mean_sb[:, tb:tb + 1], corr_sb[:],
                        axis=mybir.AxisListType.X, op=mybir.AluOpType.add)
                nc.sync.dma_start(msum_d[n][:], mean_sb[:])
    nc.compile()
    return nc


def _build_phase2(entries):
    """entries: per output block b, list of (src_block, seg_idx); seg_idx
    indexes the g stationaries tensor (NLOC, NSEG, 128, 128)."""
    nseg = max(si for segs in entries for _, si in segs) + 1
    nc = bacc.Bacc("TRN2", target_bir_lowering=False, debug=False,
                   num_devices=NCORES)
    v_d = nc.dram_tensor("v", [NLOC, L, R], _dt.bfloat16,
                         kind="ExternalInput").ap()
    g_d = nc.dram_tensor("g", [NLOC, nseg, 128, 128], _dt.bfloat16,
                         kind="ExternalInput").ap()
    out_d = nc.dram_tensor("out", [NLOC, L, R], _dt.bfloat16,
                           kind="ExternalOutput").ap()

    with tile.TileContext(nc) as tc:
        with tc.tile_pool(name="v", bufs=16) as vp, \
             tc.tile_pool(name="g", bufs=NLOC * nseg) as gp, \
             tc.tile_pool(name="o", bufs=6) as op, \
             tc.tile_pool(name="ps", bufs=6, space="PSUM") as psp:
            # v[0] first (first matmul dep), then all stationaries (tiny),
            # then the remaining v prefetch as compute proceeds.
            v0 = []
            for a in range(LB):
                t = vp.tile([128, R], _dt.bfloat16, tag="v")
                nc.sync.dma_start(t[:], v_d[0, ts(a, 128), :])
                v0.append(t)
            g_sb = [[] for _ in range(NLOC)]
            for n in range(NLOC):
                for si in range(nseg):
                    t = gp.tile([128, 128], _dt.bfloat16, tag="g")
                    nc.sync.dma_start(t[:], g_d[n, si][:])
                    g_sb[n].append(t)
            for n in range(NLOC):
                if n == 0:
                    v_sb = v0
                else:
                    v_sb = []
                    for a in range(LB):
                        t = vp.tile([128, R], _dt.bfloat16, tag="v")
                        nc.sync.dma_start(t[:], v_d[n, ts(a, 128), :])
                        v_sb.append(t)
                for b in range(LB):
                    segs = entries[b]
                    ps = psp.tile([128, R], _dt.float32, tag="ps")
                    for i, (a, si) in enumerate(segs):
                        nc.tensor.matmul(ps[:], g_sb[n][si][:], v_sb[a][:],
                                         start=(i == 0),
                                         stop=(i == len(segs) - 1))
                    o_sb = op.tile([128, R], _dt.bfloat16, tag="o")
                    nc.vector.tensor_copy(o_sb[:], ps[:])
                    nc.sync.dma_start(out_d[n, ts(b, 128), :], o_sb[:])
    nc.compile()
    return nc


_P1_CACHE = {}


def _phase1_nc(mode):
    if mode not in _P1_CACHE:
        _P1_CACHE[mode] = _build_phase1(mode)
    return _P1_CACHE[mode]


def _run(nc, in_maps, phase):
    res = run_bass_kernel_spmd(nc, in_maps, core_ids=list(range(NCORES)),
                               trace=TRACE[0])
    if TRACE[0]:
        LAST_EXEC_NS[phase] = res.exec_time_ns
    return res.results


def kernel(queries, keys, values):
    queries = np.ascontiguousarray(np.asarray(queries, dtype=np.float32))
    keys = np.ascontiguousarray(np.asarray(keys, dtype=np.float32))
    values = np.ascontiguousarray(np.asarray(values, dtype=np.float32))

    mode = P1_MODE
    store_np = np.float32 if mode == "f32r" else ml_dtypes.bfloat16
    C, Sm, A, B = _dft_mats()
    Cs = np.ascontiguousarray(C, dtype=np.float32).astype(store_np)
    Ss = np.ascontiguousarray(Sm, dtype=np.float32).astype(store_np)
    As = np.ascontiguousarray(A, dtype=np.float32).astype(store_np)
    Bs = np.ascontiguousarray(B, dtype=np.float32).astype(store_np)

    q3 = queries.reshape(N, L, R)
    k3 = keys.reshape(N, L, R)
    v3 = values.reshape(N, L, R)

    nc1 = _phase1_nc(mode)
    in_maps = []
    for c in range(NCORES):
        sl = slice(c * NLOC, (c + 1) * NLOC)
        in_maps.append({
            "q": q3[sl].astype(store_np),
            "k": k3[sl].astype(store_np),
            "cmat": Cs, "smat": Ss, "amat": As, "bmat": Bs,
        })
    res1 = _run(nc1, in_maps, 0)

    corr = np.concatenate([r["corr"] for r in res1], axis=0)  # (N, L, R) f32
    msum = np.concatenate([r["msum"] for r in res1], axis=0)  # (N, 128, LB)
    mean = msum.transpose(0, 2, 1).reshape(N, L) / R          # (N, L)

    g = mean.mean(axis=0)
    idx = np.argsort(-g, kind="stable")[:TOPK]
    w = mean[:, idx]
    e = np.exp(w - w.max(axis=1, keepdims=True))
    w = (e / e.sum(axis=1, keepdims=True)).astype(np.float32)  # (N, TOPK)

    # phase-2 stationaries: out[b*128+j] += w_k * v[(b*128+j+idx_k) mod L]
    # merged per (b, src_block); matrix content is b-independent, so dedup
    # identical segment sets across b.
    seg_of = {}
    pat = []
    entries = [[] for _ in range(LB)]
    for b in range(LB):
        acc = {}
        for kk in range(TOPK):
            sh = int(idx[kk])
            r = sh % 128
            a = ((b * 128 + sh) // 128) % LB
            acc.setdefault(a, []).append(("d1", r, kk))
            if r > 0:
                acc.setdefault((a + 1) % LB, []).append(("d2", r, kk))
        for a, parts in sorted(acc.items()):
            key = tuple(sorted(parts))
            if key not in seg_of:
                seg_of[key] = len(pat)
                pat.append(parts)
            entries[b].append((a, seg_of[key]))
    nseg = len(pat)
    gmat = np.zeros((NLOC * NCORES, nseg, 128, 128), np.float32)
    jj = np.arange(128)
    for si, parts in enumerate(pat):
        for which, r, kk in parts:
            if which == "d1":
                j = jj[: 128 - r]
                gmat[:, si, j + r, j] += w[:, kk][:, None]
            else:
                j = jj[128 - r:]
                gmat[:, si, j - (128 - r), j] += w[:, kk][:, None]
    gmat = gmat.astype(ml_dtypes.bfloat16)

    nc2 = _build_phase2(entries)
    in_maps2 = []
    for c in range(NCORES):
        sl = slice(c * NLOC, (c + 1) * NLOC)
        in_maps2.append({
            "v": v3[sl].astype(ml_dtypes.bfloat16),
            "g": gmat[sl],
        })
    res2 = _run(nc2, in_maps2, 1)
    out = np.concatenate([np.asarray(r["out"], dtype=np.float32)
                          for r in res2], axis=0)             # (N, L, R)

    out_full = out.reshape(N, L, H, E).astype(np.float32)
    corr_full = corr.reshape(N, L, H, E).astype(np.float32)
    return out_full, corr_full


# revision 12
# speedup vs baseline: 1.1368x; 1.1166x over previous
"""Trainium2 Bass kernel for nn_AutoCorrelation (8 NeuronCores, data-parallel over batch).

Algorithm (reference: AutoCorrelation block):
  corr = irfft(rfft(q, L) * conj(rfft(k, L)))        # circular cross-correlation
  top-6 delays from batch-mean of corr (mean over H,E then N)
  out  = sum_k softmax(mean[:, idx])_k * roll(v, -idx_k)

Implementation:
  - FFTs become dense DFT matmuls on the TensorEngine: rfft -> q @ C and
    q @ Sm with C[l,f]=cos(2*pi*l*f/L), Sm[l,f]=-sin(...), f=0..511, and the
    Nyquist (f=512) cosine column packed into Sm[:,0] (sin column there is 0).
    irfft -> Pre @ A + Pim @ B with the matching inverse rows (A[0]=DC row,
    B[0]=Nyquist row).
  - Phase 1 kernel (per core, 4 batch items): forward DFTs, pointwise complex
    product (VectorE), inverse DFT, corr written to HBM, per-delay row-sums
    reduced for the top-k statistic.
  - Host: tiny (N,L) mean -> top-6 indices + softmax weights.
  - Phase 2 kernel: out = sum_k w*roll(v) as PSUM-accumulated matmuls with
    w-scaled shifted-identity stationary matrices (shift along L = partition
    permutation, contracted on the TensorEngine).
"""
import math
import sys

sys.path.insert(0, "/opt/trn_rl_repo")

import numpy as np
import ml_dtypes

import concourse.bass as bass
import concourse.tile as tile
from concourse import bacc, mybir
from concourse.bass import ts
from concourse.bass_utils import run_bass_kernel_spmd

_dt = mybir.dt

N, L, H, E = 32, 1024, 8, 64
R = H * E                 # 512 rows (h,e) per batch item
NCORES = 8
NLOC = N // NCORES        # 4 batch items per core
F = 512                   # packed rfft freqs (f=0..511; Nyquist in slot 0)
TOPK = int(1.0 * math.log(L))  # 6
LB = L // 128             # 8 l/tau blocks
FB = F // 128             # 4 f blocks

# phase-1 matmul dtype: "f32r" (full fp32 precision at ~bf16 rate) or "bf16"
P1_MODE = "bf16"
TRACE = [False]           # test.py flips this to collect exec_time_ns
LAST_EXEC_NS = [0, 0]     # phase1, phase2 exec time (when TRACE)


def _dft_mats():
    l = np.arange(L)[:, None].astype(np.float64)
    f = np.arange(F)[None, :].astype(np.float64)
    ang = 2.0 * np.pi * l * f / L
    C = np.cos(ang)
    Sm = -np.sin(ang)
    Sm[:, 0] = (-1.0) ** np.arange(L)
    t = np.arange(L)[None, :].astype(np.float64)
    fa = np.arange(F)[:, None].astype(np.float64)
    ang2 = 2.0 * np.pi * fa * t / L
    A = (2.0 / L) * np.cos(ang2)
    B = -(2.0 / L) * np.sin(ang2)
    A[0, :] = 1.0 / L
    B[0, :] = (1.0 / L) * ((-1.0) ** np.arange(L))
    return C, Sm, A, B


def _build_phase1(mode):
    store = _dt.float32 if mode == "f32r" else _dt.bfloat16
    mmdt = _dt.float32r if mode == "f32r" else _dt.bfloat16

    nc = bacc.Bacc("TRN2", target_bir_lowering=False, debug=False,
                   num_devices=NCORES)
    q_d = nc.dram_tensor("q", [NLOC, L, R], store, kind="ExternalInput").ap()
    k_d = nc.dram_tensor("k", [NLOC, L, R], store, kind="ExternalInput").ap()
    c_d = nc.dram_tensor("cmat", [L, F], store, kind="ExternalInput").ap()
    s_d = nc.dram_tensor("smat", [L, F], store, kind="ExternalInput").ap()
    a_d = nc.dram_tensor("amat", [F, L], store, kind="ExternalInput").ap()
    b_d = nc.dram_tensor("bmat", [F, L], store, kind="ExternalInput").ap()
    corr_d = nc.dram_tensor("corr", [NLOC, L, R], _dt.float32,
                            kind="ExternalOutput").ap()
    msum_d = nc.dram_tensor("msum", [NLOC, 128, LB], _dt.float32,
                            kind="ExternalOutput").ap()

    def mm(ps, lhsT, rhs, start, stop):
        nc.tensor.matmul(ps, lhsT.bitcast(mmdt), rhs.bitcast(mmdt),
                         start=start, stop=stop)

    with tile.TileContext(nc) as tc:
        with tc.tile_pool(name="const", bufs=1) as cp, \
             tc.tile_pool(name="qk", bufs=12) as qk, \
             tc.tile_pool(name="pp", bufs=6) as pp, \
             tc.tile_pool(name="tmp", bufs=2) as tp, \
             tc.tile_pool(name="out", bufs=4) as op, \
             tc.tile_pool(name="ps", bufs=6, space="PSUM") as psf, \
             tc.tile_pool(name="psi", bufs=2, space="PSUM") as psi:

            # head-latency-ordered loads: the first forward chain (q @ C)
            # only needs cs[lb] + q[lb], so stream those first, then k, then
            # the sin matrices; the inverse matrices are not needed until
            # after the first full fb loop.
            # Head-latency-ordered loads, with DMA issue spread over the two
            # HWDGE engines (sync + scalar; each dma_start costs ~0.6us of
            # issue time on its sequencer). First forward chain needs cs+q.
            cs, ss, am, bm = [], [], [], []
            q0, k0 = [], []
            for lb in range(LB):
                t = cp.tile([128, F], store, tag=f"c{lb}")
                nc.sync.dma_start(t[:], c_d[ts(lb, 128), :])
                cs.append(t)
                t = qk.tile([128, R], store, tag="q")
                nc.scalar.dma_start(t[:], q_d[0, ts(lb, 128), :])
                q0.append(t)
            for lb in range(LB):
                t = qk.tile([128, R], store, tag="k")
                nc.scalar.dma_start(t[:], k_d[0, ts(lb, 128), :])
                k0.append(t)
                t = cp.tile([128, F], store, tag=f"s{lb}")
                nc.sync.dma_start(t[:], s_d[ts(lb, 128), :])
                ss.append(t)
            for fb in range(FB):
                t = cp.tile([128, L], store, tag=f"a{fb}")
                nc.sync.dma_start(t[:], a_d[ts(fb, 128), :])
                am.append(t)
                t = cp.tile([128, L], store, tag=f"b{fb}")
                nc.sync.dma_start(t[:], b_d[ts(fb, 128), :])
                bm.append(t)

            for n in range(NLOC):
                if n == 0:
                    q_sb, k_sb = q0, k0
                else:
                    q_sb, k_sb = [], []
                    for lb in range(LB):
                        t = qk.tile([128, R], store, tag="q")
                        nc.sync.dma_start(t[:], q_d[n, ts(lb, 128), :])
                        q_sb.append(t)
                        t = qk.tile([128, R], store, tag="k")
                        nc.scalar.dma_start(t[:], k_d[n, ts(lb, 128), :])
                        k_sb.append(t)

                pre_sb, pim_sb = [], []
                for fb in range(FB):
                    ps_qre = psf.tile([128, R], _dt.float32, tag="fwd")
                    ps_qim = psf.tile([128, R], _dt.float32, tag="fwd")
                    ps_kre = psf.tile([128, R], _dt.float32, tag="fwd")
                    ps_kim = psf.tile([128, R], _dt.float32, tag="fwd")
                    # chain order matches head DMA arrival: cs+q, cs+k, ss+q, ss+k
                    for lb in range(LB):
                        mm(ps_qre[:], cs[lb][:, ts(fb, 128)], q_sb[lb][:],
                           lb == 0, lb == LB - 1)
                    for lb in range(LB):
                        mm(ps_kre[:], cs[lb][:, ts(fb, 128)], k_sb[lb][:],
                           lb == 0, lb == LB - 1)
                    for lb in range(LB):
                        mm(ps_qim[:], ss[lb][:, ts(fb, 128)], q_sb[lb][:],
                           lb == 0, lb == LB - 1)
                    for lb in range(LB):
                        mm(ps_kim[:], ss[lb][:, ts(fb, 128)], k_sb[lb][:],
                           lb == 0, lb == LB - 1)

                    qre = tp.tile([128, R], _dt.float32, tag="qre")
                    qim = tp.tile([128, R], _dt.float32, tag="qim")
                    nc.scalar.mul(qre[:], ps_qre[:], 1.0)
                    nc.scalar.mul(qim[:], ps_qim[:], 1.0)
                    t1 = tp.tile([128, R], _dt.float32, tag="t1")
                    t2 = tp.tile([128, R], _dt.float32, tag="t2")
                    nc.vector.tensor_mul(t1[:], qre[:], ps_kre[:])
                    nc.vector.tensor_mul(t2[:], qim[:], ps_kim[:])
                    pre = pp.tile([128, R], store, tag="pre")
                    nc.vector.tensor_add(pre[:], t1[:], t2[:])
                    t3 = tp.tile([128, R], _dt.float32, tag="t3")
                    t4 = tp.tile([128, R], _dt.float32, tag="t4")
                    nc.vector.tensor_mul(t3[:], qim[:], ps_kre[:])
                    nc.vector.tensor_mul(t4[:], qre[:], ps_kim[:])
                    pim = pp.tile([128, R], store, tag="pim")
                    nc.vector.tensor_sub(pim[:], t3[:], t4[:])
                    if fb == 0:
                        # slot-0 carries DC (cos col) and Nyquist (packed in
                        # Sm col 0): Pre[0] = Qre0*Kre0 (= t1 row0),
                        # Pim[0] = Qim0*Kim0 (= t2 row0)
                        nc.vector.tensor_copy(pre[0:1, :], t1[0:1, :])
                        nc.vector.tensor_copy(pim[0:1, :], t2[0:1, :])
                    pre_sb.append(pre)
                    pim_sb.append(pim)

                mean_sb = op.tile([128, LB], _dt.float32, tag="mean")
                for tb in range(LB):
                    ps_c = psi.tile([128, R], _dt.float32, tag="inv")
                    first = True
                    for fb in range(FB):
                        mm(ps_c[:], am[fb][:, ts(tb, 128)], pre_sb[fb][:],
                           first, False)
                        first = False
                        mm(ps_c[:], bm[fb][:, ts(tb, 128)], pim_sb[fb][:],
                           False, fb == FB - 1)
                    corr_sb = op.tile([128, R], _dt.float32, tag="corr")
                    nc.vector.tensor_copy(corr_sb[:], ps_c[:])
                    (nc.scalar if tb % 2 else nc.sync).dma_start(
                        corr_d[n, ts(tb, 128), :], corr_sb[:])
                    nc.vector.tensor_reduce(
                        mean_sb[:, tb:tb + 1], corr_sb[:],
                        axis=mybir.AxisListType.X, op=mybir.AluOpType.add)
                nc.sync.dma_start(msum_d[n][:], mean_sb[:])
    nc.compile()
    return nc


def _build_phase2(entries):
    """entries: per output block b, list of (src_block, seg_idx); seg_idx
    indexes the g stationaries tensor (NLOC, NSEG, 128, 128)."""
    nseg = max(si for segs in entries for _, si in segs) + 1
    nc = bacc.Bacc("TRN2", target_bir_lowering=False, debug=False,
                   num_devices=NCORES)
    v_d = nc.dram_tensor("v", [NLOC, L, R], _dt.bfloat16,
                         kind="ExternalInput").ap()
    # g is host-packed as (NLOC, 128, nseg*128): one contiguous DMA per n;
    # stationary si is the [:, si*128:(si+1)*128] slice.
    g_d = nc.dram_tensor("g", [NLOC, 128, nseg * 128], _dt.bfloat16,
                         kind="ExternalInput").ap()
    out_d = nc.dram_tensor("out", [NLOC, L, R], _dt.bfloat16,
                           kind="ExternalOutput").ap()

    with tile.TileContext(nc) as tc:
        with tc.tile_pool(name="v", bufs=16) as vp, \
             tc.tile_pool(name="g", bufs=NLOC) as gp, \
             tc.tile_pool(name="o", bufs=6) as op, \
             tc.tile_pool(name="ps", bufs=8, space="PSUM") as psp:
            # v[0] first (first matmul dep), then the stationaries (tiny),
            # then the remaining v prefetch as compute proceeds.
            v0 = []
            for a in range(LB):
                t = vp.tile([128, R], _dt.bfloat16, tag="v")
                (nc.scalar if a % 2 else nc.sync).dma_start(
                    t[:], v_d[0, ts(a, 128), :])
                v0.append(t)
            g_sb = []
            for n in range(NLOC):
                t = gp.tile([128, nseg * 128], _dt.bfloat16, tag="g")
                nc.sync.dma_start(t[:], g_d[n][:])
                g_sb.append(t)
            for n in range(NLOC):
                if n == 0:
                    v_sb = v0
                else:
                    v_sb = []
                    for a in range(LB):
                        t = vp.tile([128, R], _dt.bfloat16, tag="v")
                        (nc.scalar if a % 2 else nc.sync).dma_start(
                            t[:], v_d[n, ts(a, 128), :])
                        v_sb.append(t)
                for b in range(LB):
                    segs = entries[b]
                    ps = psp.tile([128, R], _dt.float32, tag="ps")
                    for i, (a, si) in enumerate(segs):
                        nc.tensor.matmul(ps[:], g_sb[n][:, ts(si, 128)],
                                         v_sb[a][:],
                                         start=(i == 0),
                                         stop=(i == len(segs) - 1))
                    o_sb = op.tile([128, R], _dt.bfloat16, tag="o")
                    nc.vector.tensor_copy(o_sb[:], ps[:])
                    (nc.scalar if b % 2 else nc.sync).dma_start(
                        out_d[n, ts(b, 128), :], o_sb[:])
    nc.compile()
    return nc


_P1_CACHE = {}


def _phase1_nc(mode):
    if mode not in _P1_CACHE:
        _P1_CACHE[mode] = _build_phase1(mode)
    return _P1_CACHE[mode]


def _run(nc, in_maps, phase):
    res = run_bass_kernel_spmd(nc, in_maps, core_ids=list(range(NCORES)),
                               trace=TRACE[0])
    if TRACE[0]:
        LAST_EXEC_NS[phase] = res.exec_time_ns
    return res.results


def kernel(queries, keys, values):
    queries = np.ascontiguousarray(np.asarray(queries, dtype=np.float32))
    keys = np.ascontiguousarray(np.asarray(keys, dtype=np.float32))
    values = np.ascontiguousarray(np.asarray(values, dtype=np.float32))

    mode = P1_MODE
    store_np = np.float32 if mode == "f32r" else ml_dtypes.bfloat16
    C, Sm, A, B = _dft_mats()
    Cs = np.ascontiguousarray(C, dtype=np.float32).astype(store_np)
    Ss = np.ascontiguousarray(Sm, dtype=np.float32).astype(store_np)
    As = np.ascontiguousarray(A, dtype=np.float32).astype(store_np)
    Bs = np.ascontiguousarray(B, dtype=np.float32).astype(store_np)

    q3 = queries.reshape(N, L, R)
    k3 = keys.reshape(N, L, R)
    v3 = values.reshape(N, L, R)

    nc1 = _phase1_nc(mode)
    in_maps = []
    for c in range(NCORES):
        sl = slice(c * NLOC, (c + 1) * NLOC)
        in_maps.append({
            "q": q3[sl].astype(store_np),
            "k": k3[sl].astype(store_np),
            "cmat": Cs, "smat": Ss, "amat": As, "bmat": Bs,
        })
    res1 = _run(nc1, in_maps, 0)

    corr = np.concatenate([r["corr"] for r in res1], axis=0)  # (N, L, R) f32
    msum = np.concatenate([r["msum"] for r in res1], axis=0)  # (N, 128, LB)
    mean = msum.transpose(0, 2, 1).reshape(N, L) / R          # (N, L)

    g = mean.mean(axis=0)
    idx = np.argsort(-g, kind="stable")[:TOPK]
    w = mean[:, idx]
    e = np.exp(w - w.max(axis=1, keepdims=True))
    w = (e / e.sum(axis=1, keepdims=True)).astype(np.float32)  # (N, TOPK)

    # phase-2 stationaries: out[b*128+j] += w_k * v[(b*128+j+idx_k) mod L]
    # merged per (b, src_block); matrix content is b-independent, so dedup
    # identical segment sets across b.
    seg_of = {}
    pat = []
    entries = [[] for _ in range(LB)]
    for b in range(LB):
        acc = {}
        for kk in range(TOPK):
            sh = int(idx[kk])
            r = sh % 128
            a = ((b * 128 + sh) // 128) % LB
            acc.setdefault(a, []).append(("d1", r, kk))
            if r > 0:
                acc.setdefault((a + 1) % LB, []).append(("d2", r, kk))
        for a, parts in sorted(acc.items()):
            key = tuple(sorted(parts))
            if key not in seg_of:
                seg_of[key] = len(pat)
                pat.append(parts)
            entries[b].append((a, seg_of[key]))
    nseg = len(pat)
    gmat = np.zeros((NLOC * NCORES, nseg, 128, 128), np.float32)
    jj = np.arange(128)
    for si, parts in enumerate(pat):
        for which, r, kk in parts:
            if which == "d1":
                j = jj[: 128 - r]
                gmat[:, si, j + r, j] += w[:, kk][:, None]
            else:
                j = jj[128 - r:]
                gmat[:, si, j - (128 - r), j] += w[:, kk][:, None]
    # pack (NLOC, nseg, 128, 128) -> (NLOC, 128, nseg*128) for 1-DMA-per-n
    gmat = np.ascontiguousarray(
        gmat.transpose(0, 2, 1, 3).reshape(NLOC * NCORES, 128, nseg * 128)
    ).astype(ml_dtypes.bfloat16)

    nc2 = _build_phase2(entries)
    in_maps2 = []
    for c in range(NCORES):
        sl = slice(c * NLOC, (c + 1) * NLOC)
        in_maps2.append({
            "v": v3[sl].astype(ml_dtypes.bfloat16),
            "g": gmat[sl],
        })
    res2 = _run(nc2, in_maps2, 1)
    out = np.concatenate([np.asarray(r["out"], dtype=np.float32)
                          for r in res2], axis=0)             # (N, L, R)

    out_full = out.reshape(N, L, H, E).astype(np.float32)
    corr_full = corr.reshape(N, L, H, E).astype(np.float32)
    return out_full, corr_full
